# revision 1
# baseline (speedup 1.0000x reference)
"""Baichuan attention (B=2, S=1024, H=5120, NH=40, fp32) on 8 trn2 NeuronCores.

Strategy: tensor-parallel over heads (5 heads/core). Each core computes
qkv^T for its heads (fp16 matmuls, fp32 PSUM accumulate), causal+alibi
attention without max-subtraction (exp args are small; probs scaled by
1/64 to stay in fp16 range), and a partial o_proj over its 640
contraction dims. The 8 partial outputs are summed on the host.

The alibi mask is never shipped: slopes are derived from the mask input
on the host (mask[h, q, k] = causal + slope_h * k) and turned into
per-partition bias vectors for the exp activation; causality is handled
by only computing k-tiles at or below the diagonal plus a triangular
-1e30 mask on the diagonal tile.

The PE runs only real matmuls: the per-q softmax stabilizer is a DVE
scalar_tensor_tensor (q-ramp times per-head slope added into PSUM), and
the softmax normalizer Z is accumulated tile-wise on DVE then reduced
across partitions on the otherwise-idle GpSimd engine, whose all-reduce
output is already partition-broadcast (so no PE broadcast matmul).

All device-side layouts put the matmul contraction dim on partitions:
  xt    [B, 128, KT, S]    x^T tiles  (partition = hidden dim within k-tile)
  wqkv  [3*HPC, 128, KT, 128]  W_pack^T strips per output m-tile
  wo    [HPC, 128, H]      W_o^T strips (partition = per-core contraction dim)
  out   [B*QT, 128, H]     partial output, fp16 (token tiles on partitions)
"""

import math
from contextlib import ExitStack
from dataclasses import dataclass

import numpy as np

import concourse.bass as bass
import concourse.bass_isa as bass_isa
import concourse.mybir as mybir
from concourse import bacc
import concourse.tile as tile
from concourse import masks
from concourse.bass_utils import run_bass_kernel_spmd

F16 = mybir.dt.float16
F32 = mybir.dt.float32
P = 128
NEG = -60000.0
SCALE = 1.0 / math.sqrt(128.0)
LN_PSCALE = math.log(64.0)  # probs scaled by 1/64 so fp16 never overflows


@dataclass(frozen=True)
class Cfg:
    B: int = 2
    S: int = 1024
    KT: int = 40  # contraction tiles; H = KT * 128
    HPC: int = 5  # heads per core
    n_cores: int = 8

    @property
    def H(self):
        return self.KT * P

    @property
    def QT(self):
        return self.S // P

    @property
    def MQKV(self):
        return 3 * self.HPC

    @property
    def NBLK(self):
        return self.S // 512

    @property
    def OC(self):
        return self.H // 512


FULL = Cfg()


def build_nc(cfg: Cfg) -> bass.Bass:
    nc = bacc.Bacc("TRN2", debug=False)
    B, S, KT, HPC, QT, MQKV = cfg.B, cfg.S, cfg.KT, cfg.HPC, cfg.QT, cfg.MQKV

    xt_d = nc.dram_tensor("xt", [B, P, KT, S], F16, kind="ExternalInput")
    ws_d = nc.dram_tensor("wqkv", [MQKV, P, KT, P], F16, kind="ExternalInput")
    wo_d = nc.dram_tensor("wo", [HPC, P, cfg.H], F16, kind="ExternalInput")
    bias_d = nc.dram_tensor("bias", [P, HPC * QT], F32, kind="ExternalInput")
    qrbc_d = nc.dram_tensor("qrbc", [P, S], F16, kind="ExternalInput")
    slcp_d = nc.dram_tensor("slcp", [P, HPC], F32, kind="ExternalInput")
    out_d = nc.dram_tensor("out", [B * QT, P, cfg.H], F16, kind="ExternalOutput")

    with ExitStack() as ctx:
        tc = ctx.enter_context(tile.TileContext(nc))
        ctx.enter_context(
            nc.allow_low_precision(
                reason="fp16 Z/partial-out staging is within tolerance; "
                "PSUM accumulation is fp32 in-bank"
            )
        )
        consts = ctx.enter_context(tc.tile_pool(name="consts", bufs=1))
        xt_pool = ctx.enter_context(tc.tile_pool(name="xt", bufs=1))
        wqkv_pool = ctx.enter_context(tc.tile_pool(name="wqkv", bufs=2))
        qkvt_pool = ctx.enter_context(tc.tile_pool(name="qkvt", bufs=2))
        v_pool = ctx.enter_context(tc.tile_pool(name="v", bufs=4))
        p_pool = ctx.enter_context(tc.tile_pool(name="p", bufs=4))
        attnt_pool = ctx.enter_context(tc.tile_pool(name="attnt", bufs=2))
        atn_pool = ctx.enter_context(tc.tile_pool(name="atn", bufs=3))
        zacc_pool = ctx.enter_context(tc.tile_pool(name="zacc", bufs=2))
        vt_pool = ctx.enter_context(tc.tile_pool(name="vt", bufs=1))
        wo_pool = ctx.enter_context(tc.tile_pool(name="wo", bufs=3 * HPC))
        out_pool = ctx.enter_context(tc.tile_pool(name="out", bufs=4))
        mm_pool = ctx.enter_context(tc.tile_pool(name="mm", bufs=3, space="PSUM"))
        sc_pool = ctx.enter_context(tc.tile_pool(name="sc", bufs=3, space="PSUM"))
        acc_pool = ctx.enter_context(tc.tile_pool(name="acc", bufs=2, space="PSUM"))

        # constants
        ident = consts.tile([P, P], F16)
        masks.make_identity(nc, ident[:])
        tri = consts.tile([P, P], F16)
        # tri[k, q] = NEG where k > q (strictly below diagonal), else 0
        nc.gpsimd.memset(tri[:], 0.0)
        nc.gpsimd.affine_select(
            out=tri[:],
            in_=tri[:],
            compare_op=mybir.AluOpType.is_ge,
            fill=NEG,
            base=0,
            # keep where (q - k) >= 0, fill NEG where k > q
            pattern=[[1, P]],
            channel_multiplier=-1,
        )
        bias_sb = consts.tile([P, HPC * QT], F32)
        nc.sync.dma_start(bias_sb[:], bias_d[:])
        qrbc_sb = consts.tile([P, S], F16)
        nc.sync.dma_start(qrbc_sb[:], qrbc_d[:])
        slcp_sb = consts.tile([P, HPC], F32)
        nc.sync.dma_start(slcp_sb[:], slcp_d[:])

        # PE warm-up: self-contained matmuls on the identity tile keep the
        # PE busy through the p-state ramp while input DMAs stream
        warm = mm_pool.tile([P, 512], F32, tag="mm", name="warm")
        for _ in range(185):
            nc.tensor.matmul(warm[:, :P], ident[:], ident[:], start=True, stop=True)

        # last k-tile index contributing to each 512-wide q block
        def i_last(blk):
            return min(QT - 1, (blk + 1) * 4 - 1)

        if KT >= 8:
            sizes = [1, 2, 4, 5]
            rem = KT - sum(sizes)
            nrem = 4
            q, r = divmod(rem, nrem)
            sizes += [q + (1 if i < r else 0) for i in range(nrem)]
        else:
            sizes = [1] * KT
        k2chunk = []
        for ci, s in enumerate(sizes):
            for j in range(s):
                k2chunk.append((ci, j))
        state = {}

        chunk_c0 = []
        c0 = 0
        for s in sizes:
            chunk_c0.append(c0)
            c0 += s
        # k index after whose matmuls chunk ci has no further readers in
        # this m-tile (used to prefetch the next batch's chunk)
        chunk_last_k = [c + s - 1 for c, s in zip(chunk_c0, sizes)]

        def load_xt_chunk(b, ci):
            s = sizes[ci]
            xc = xt_pool.tile([P, s, S], F16, tag=f"xt{ci}", name=f"xt{ci}")
            nc.sync.dma_start(xc[:], xt_d[b, :, chunk_c0[ci] : chunk_c0[ci] + s, :])
            state.setdefault((b, "xtc"), {})[ci] = xc
            if len(state[b, "xtc"]) == len(sizes):
                state[b, "xt"] = [state[b, "xtc"][i] for i in range(len(sizes))]

        def load_xt(b):
            # chunk tiles with progressive sizes: QKV starts as soon as the
            # first small chunk lands instead of after the full 10MB
            for ci in range(len(sizes)):
                load_xt_chunk(b, ci)

        def prefetch_ws(b, m):
            ws = wqkv_pool.tile([P, KT, P], F16, tag="ws", name=f"ws{b}_{m}")
            nc.sync.dma_start(ws[:], ws_d[m])
            state[b, "ws", m] = ws

        def qkv_mtile(b, m, chunk_hook=None):
            # one 128-row strip of qkv^T = W^T.T @ x^T (contraction over H)
            if (b, "qkvt") not in state:
                state[b, "qkvt"] = qkvt_pool.tile(
                    [P, 2 * HPC, S], F16, tag="qkvt", name=f"qkvt{b}"
                )
            qkvt_sb = state[b, "qkvt"]
            xt_ch = state[b, "xt"]
            if (b, "ws", m) in state:
                ws = state.pop((b, "ws", m))
            else:
                ws = wqkv_pool.tile([P, KT, P], F16, tag="ws", name=f"ws{b}_{m}")
                nc.sync.dma_start(ws[:], ws_d[m])
            ps = [
                mm_pool.tile([P, 512], F32, tag="mm", name=f"ps{hf}")
                for hf in range(S // 512)
            ]
            for k in range(KT):
                for hf in range(S // 512):
                    nc.tensor.matmul(
                        ps[hf][:],
                        ws[:, k, :],
                        xt_ch[k2chunk[k][0]][:, k2chunk[k][1], hf * 512 : (hf + 1) * 512],
                        start=(k == 0),
                        stop=(k == KT - 1),
                    )
                if chunk_hook is not None and k == chunk_last_k[k2chunk[k][0]]:
                    chunk_hook(k2chunk[k][0])
                if b == 0 and m == 0 and k == chunk_last_k[k2chunk[k][0]] and k < KT - 1:
                    # first pass races the x^T DMA: keep the PE warm across
                    # the chunk-arrival gaps (sc pool is idle until attention)
                    fl = sc_pool.tile([P, 512], F32, tag="sc", name="fill")
                    for _ in range(10):
                        nc.tensor.matmul(
                            fl[:, :P], ident[:], ident[:], start=True, stop=True
                        )
                if k % 3 == 2:
                    yield
            if m < 2 * HPC:
                # PSUM->SBUF staging on the Scalar engine (Copy shares the
                # act table with Exp): keeps the DVE free for attention work
                for hf in range(S // 512):
                    nc.scalar.activation(
                        qkvt_sb[:, m, hf * 512 : (hf + 1) * 512],
                        ps[hf][:],
                        mybir.ActivationFunctionType.Copy,
                    )
            else:
                # v^T strip: stage, then PE-transpose to per-head natural V
                hh = m - 2 * HPC
                vt = vt_pool.tile([P, S], F16, tag="vt", name=f"vt{b}_{hh}")
                for hf in range(S // 512):
                    nc.scalar.activation(
                        vt[:, hf * 512 : (hf + 1) * 512],
                        ps[hf][:],
                        mybir.ActivationFunctionType.Copy,
                    )
                v_sb = v_pool.tile([P, QT, P], F16, tag="v", name=f"v{b}_{hh}")
                state[b, "v", hh] = v_sb
                for i in range(QT):
                    tp = mm_pool.tile([P, P], F16, tag="mm")
                    nc.tensor.transpose(tp[:], vt[:, i * P : (i + 1) * P], ident[:])
                    nc.vector.tensor_copy(v_sb[:, i, :], tp[:])
                    if i % 4 == 3:
                        yield

        def attn_head(b, hh, zacc_ref, zeng=None):
            # scores^T = K^T.T @ Q^T with k-positions on partitions; causal
            # ragged tiles; p = exp(s/sqrt(d) + alibi_k - slope*q - ln64)
            if (b, "attnt") not in state:
                state[b, "attnt"] = attnt_pool.tile(
                    [P, HPC, S], F16, tag="attnt", name=f"attnt{b}"
                )
            attnt_sb = state[b, "attnt"]
            qkvt_sb = state[b, "qkvt"]
            v_sb = state[b, "v", hh]
            at = [
                acc_pool.tile([P, 512], F32, tag="acc", name=f"at{blk}")
                for blk in range(cfg.NBLK)
            ]
            zacc = [
                zacc_pool.tile([P, 512], F32, tag=f"zacc{blk}", name=f"z{b}_{hh}_{blk}")
                for blk in range(cfg.NBLK)
            ]
            zacc_ref["zacc"] = zacc

            def pv_stage(i, blk, off, w, pt):
                # Z partial sums: tile-wise accumulate (partition reduction
                # happens once per block on GpSimd at the end)
                if i == 0:
                    zeng.tensor_copy(zacc[blk][:, off : off + w], pt[:, :w])
                else:
                    zeng.tensor_tensor(
                        zacc[blk][:, off : off + w],
                        zacc[blk][:, off : off + w],
                        pt[:, :w],
                        mybir.AluOpType.add,
                    )
                nc.tensor.matmul(
                    at[blk][:, off : off + w],
                    v_sb[:, i, :],
                    pt[:, :w],
                    start=(i == 0),
                    stop=(i == i_last(blk)),
                )
                if i == i_last(blk):
                    # evacuate finished PV accumulator to SBUF right away so
                    # the PSUM bank frees for the next head without waiting
                    # on the Z reduction chain, and kick off the Z partition
                    # reduction on GpSimd immediately so its ~2.5us latency
                    # hides under the remaining tiles
                    atn = atn_pool.tile(
                        [P, 512], F16, tag="atn", name=f"atn{b}_{hh}_{blk}"
                    )
                    state[b, "atn", hh, blk] = atn
                    nc.vector.tensor_copy(atn[:], at[blk][:])
                    nc.gpsimd.partition_all_reduce(
                        zacc[blk][:],
                        zacc[blk][:],
                        channels=P,
                        reduce_op=bass_isa.ReduceOp.add,
                    )

            # software-pipelined by one tile: tile n's PV matmul is emitted
            # after tile n+1's score matmul, so the PE's in-order queue never
            # parks on the stabilizer->exp chain before filler can run
            pv_prev = None
            for i in range(QT):
                k0 = i * P
                for blk in range(cfg.NBLK):
                    c0 = max(blk * 512, k0)
                    c1 = (blk + 1) * 512
                    if c0 >= c1:
                        continue  # q block entirely above the diagonal
                    w = c1 - c0
                    sc = sc_pool.tile([P, 512], F32, tag="sc")
                    nc.tensor.matmul(
                        sc[:, :w],
                        qkvt_sb[:, HPC + hh, k0 : k0 + P],
                        qkvt_sb[:, hh, c0:c1],
                        start=True,
                        stop=True,
                    )
                    # per-q stabilizer: scores += -slope*q*sqrt(d) (any per-q
                    # shift cancels in the softmax normalization); DVE fused
                    # multiply-add keeps this off the PE
                    nc.vector.scalar_tensor_tensor(
                        out=sc[:, :w],
                        in0=qrbc_sb[:, c0:c1],
                        scalar=slcp_sb[:, hh : hh + 1],
                        in1=sc[:, :w],
                        op0=mybir.AluOpType.mult,
                        op1=mybir.AluOpType.add,
                    )
                    if c0 == k0:  # diagonal tile: causal triangle
                        nc.vector.tensor_tensor(
                            sc[:, :P], sc[:, :P], tri[:], mybir.AluOpType.add
                        )
                    pt = p_pool.tile([P, 512], F16, tag="p")
                    nc.scalar.activation(
                        pt[:, :w],
                        sc[:, :w],
                        mybir.ActivationFunctionType.Exp,
                        bias=bias_sb[:, hh * QT + i : hh * QT + i + 1],
                        scale=SCALE,
                    )
                    if pv_prev is not None:
                        pv_stage(*pv_prev)
                    pv_prev = (i, blk, c0 - blk * 512, w, pt)
                    yield
            # leave the final tile's PV to the caller: flushing it after the
            # NEXT head's first score matmul gives its exp chain ~2.5us of
            # breathing room instead of stalling the PE at the head boundary
            zacc_ref["flush"] = lambda: pv_stage(*pv_prev)

        def attn_tail_a(b, hh, zacc):
            # 1/Z: reciprocal of a single partition row (DVE reciprocal on a
            # full 128x512 tile costs ~4us and head-of-line blocks the DVE
            # queue), then partition-broadcast it on the idle GpSimd
            for blk in range(cfg.NBLK):
                nc.vector.reciprocal_approx_fast(
                    out=zacc[blk][0:1, :], in_=zacc[blk][0:1, :]
                )
            for blk in range(cfg.NBLK):
                nc.gpsimd.partition_broadcast(
                    zacc[blk][:], zacc[blk][0:1, :], channels=P
                )

        def attn_tail_b(b, hh, zacc):
            # normalize: attnT = attnT_unnorm * (1/Z); pure SBUF DVE multiply
            attnt_sb = state[b, "attnt"]
            for blk in range(cfg.NBLK):
                nc.vector.tensor_tensor(
                    attnt_sb[:, hh, blk * 512 : (blk + 1) * 512],
                    state.pop((b, "atn", hh, blk))[:],
                    zacc[blk][:],
                    mybir.AluOpType.mult,
                )

        def attn_stream(b, zeng):
            # run the 5 heads back to back; each head's normalize chain runs
            # deferred in two stages during the NEXT head so neither the
            # GpSimd partition reduction nor the broadcast ever stalls the
            # DVE queue that feeds the stabilizer->exp->PV pipeline
            pending = None
            flush = None
            for hh in range(HPC):
                zacc_ref = {}
                g = attn_head(b, hh, zacc_ref, zeng)
                for j, _ in enumerate(g):
                    yield
                    if j == 0 and flush is not None:
                        flush()
                        flush = None
                    if pending is not None:
                        if j == 4:
                            attn_tail_a(*pending)
                        elif j == 8:
                            attn_tail_b(*pending)
                            pending = None
                pending = (b, hh, zacc_ref["zacc"])
                flush = zacc_ref["flush"]
            if flush is not None:
                flush()
            if pending is not None:
                attn_tail_a(*pending)
                attn_tail_b(*pending)

        wo_cache = {}  # oc -> wos; at most 3 chunks resident (bufs = 3*HPC)

        def load_wo(oc):
            # W_o strips are batch-independent; cache by chunk so the second
            # batch's o_proj can reuse whatever is still resident
            if oc in wo_cache:
                return wo_cache[oc]
            while len(wo_cache) >= 3:
                # the pool recycles the oldest buffers; drop its cache entry
                wo_cache.pop(next(iter(wo_cache)))
            wos = []
            for k in range(HPC):
                wt = wo_pool.tile([P, 512], F16, tag="wo", name=f"wo{oc}_{k}")
                nc.sync.dma_start(wt[:], wo_d[k, :, oc * 512 : (oc + 1) * 512])
                wos.append(wt)
            wo_cache[oc] = wos
            return wos

        def oproj_chunk(b, oc, prefetch=None, pools=None):
            # out[t, oc] partial: contraction over this core's 5*128 dims
            attnt_sb = state[b, "attnt"]
            wos = load_wo(oc)
            pools = pools or [(mm_pool, "mm")]
            for t in range(QT):
                if t == 1 and prefetch is not None:
                    load_wo(prefetch)
                pool, ptag = pools[t % len(pools)]
                po = pool.tile([P, 512], F32, tag=ptag)
                for k in range(HPC):
                    nc.tensor.matmul(
                        po[:],
                        attnt_sb[:, k, t * P : (t + 1) * P],
                        wos[k][:],
                        start=(k == 0),
                        stop=(k == HPC - 1),
                    )
                # stage to fp16 on the Scalar engine (Copy shares the act
                # table with Exp, so no table reloads) to keep the DVE free
                ot = out_pool.tile([P, 512], F16, tag="ot")
                nc.scalar.activation(
                    ot[:], po[:], mybir.ActivationFunctionType.Copy
                )
                nc.sync.dma_start(
                    out_d[b * QT + t, :, oc * 512 : (oc + 1) * 512], ot[:]
                )
                yield

        def drain(gens):
            for g in gens:
                for _ in g:
                    pass

        def interleave(a_gens, b_gens, ratio):
            """Step generator stream a, inserting `ratio` steps of stream b
            after each a-step. Instruction-level pipelining: b's big dense
            matmuls fill a's dependency stalls so the PE never idles long
            enough for HAM to re-throttle."""
            bi = 0
            for g in a_gens:
                for _ in g:
                    n = 0
                    while n < ratio and bi < len(b_gens):
                        try:
                            next(b_gens[bi])
                            n += 1
                        except StopIteration:
                            bi += 1
            drain(b_gens[bi:])

        # ---- software pipeline: keep the PE stream dense so HAM stays warm
        prefetch_ws(0, 0)
        load_xt(0)
        drain([qkv_mtile(0, m) for m in range(MQKV)])
        load_xt(1)
        interleave(
            [attn_stream(0, nc.vector)],
            [qkv_mtile(1, m) for m in range(MQKV)],
            ratio=3,
        )
        interleave(
            [attn_stream(1, nc.vector)],
            [
                oproj_chunk(0, oc, prefetch=(oc + 1) if oc + 1 < cfg.OC else 0)
                for oc in range(cfg.OC)
            ],
            ratio=1,
        )
        drain(
            [
                oproj_chunk(
                    1,
                    oc,
                    prefetch=(oc + 1) if oc + 1 < cfg.OC else None,
                    pools=[(mm_pool, "mm"), (sc_pool, "sc")],
                )
                for oc in range(cfg.OC)
            ]
        )

    nc.compile()
    return nc


def prep_inputs(hidden_states, W_pack, W_o, attention_mask, cfg: Cfg = FULL):
    """Shard + lay out the full inputs for the 8 cores. Returns in_maps."""
    B, S, KT, HPC = cfg.B, cfg.S, cfg.KT, cfg.HPC
    H = cfg.H
    hs = np.asarray(hidden_states)
    wp = np.asarray(W_pack)
    wo = np.asarray(W_o)
    am = np.asarray(attention_mask)

    # x^T layout [B, 128, KT, S]: xt[b, p, k, t] = hs[b, t, k*128 + p]
    xt = np.ascontiguousarray(
        hs.reshape(B, S, KT, P).transpose(0, 3, 2, 1).astype(np.float16)
    )

    # alibi slopes from the mask: mask[h, q, k] = causal + slope_h * k
    slopes = am[:, -1, 1].astype(np.float64)  # mask[h, S-1, 1] = slope_h

    # q-position ramp, identical in every partition (exact in fp16)
    qrbc = np.tile(np.arange(S, dtype=np.float16)[None, :], (P, 1))

    kvec = np.arange(P, dtype=np.float64)
    in_maps = []
    for c in range(cfg.n_cores):
        heads = range(c * HPC, (c + 1) * HPC)
        # W_pack^T strips: m-tiles [q0..q4, k0..k4, v0..v4] for this core's heads
        rows = []
        for sec in range(3):  # q, k, v blocks of W_pack
            for h in heads:
                r0 = sec * H + h * P
                rows.append(wp[r0 : r0 + P, :])  # [128, H]
        # strip[m, p, k, j] = W_pack[row_j, k*128 + p]
        ws = np.stack(
            [r.T.reshape(KT, P, P).transpose(1, 0, 2) for r in rows]
        ).astype(np.float16)

        # W_o^T strip: wo_c[k, p, o] = W_o[o, c*HPC*128 + k*128 + p]
        wo_c = np.ascontiguousarray(
            wo[:, c * HPC * P : (c + 1) * HPC * P].T.reshape(HPC, P, H)
        ).astype(np.float16)

        # exp bias table [128, HPC*QT]: col hh*QT + i -> slope*(i*128+k) - lnPS
        bias = np.empty((P, HPC * cfg.QT), dtype=np.float32)
        # stabilizer slope column: -slope_h * sqrt(d), replicated down partitions
        slcp = np.empty((P, HPC), dtype=np.float32)
        for hh, h in enumerate(heads):
            for i in range(cfg.QT):
                bias[:, hh * cfg.QT + i] = (
                    slopes[h] * (i * P + kvec) - LN_PSCALE
                ).astype(np.float32)
            slcp[:, hh] = np.float32(-slopes[h] * math.sqrt(128.0))

        in_maps.append(
            {
                "xt": xt,
                "wqkv": np.ascontiguousarray(ws),
                "wo": wo_c,
                "bias": bias,
                "qrbc": qrbc,
                "slcp": slcp,
            }
        )
    return in_maps


_CACHE = {}


def _get_nc(cfg: Cfg = FULL) -> bass.Bass:
    if cfg not in _CACHE:
        _CACHE[cfg] = build_nc(cfg)
    return _CACHE[cfg]


def run(hidden_states, W_pack, W_o, attention_mask, cfg: Cfg = FULL, **kw):
    nc = _get_nc(cfg)
    in_maps = prep_inputs(hidden_states, W_pack, W_o, attention_mask, cfg)
    res = run_bass_kernel_spmd(nc, in_maps, core_ids=list(range(cfg.n_cores)), **kw)
    # sum the per-core partials (fp16 -> fp32), unshard to [B, S, H]
    acc = np.zeros((cfg.B * cfg.QT, P, cfg.H), dtype=np.float32)
    for r in res.results:
        acc += r["out"].astype(np.float32)
    out = acc.reshape(cfg.B, cfg.S, cfg.H)
    return out, res


def kernel(hidden_states, W_pack, W_o, attention_mask):
    out, _ = run(hidden_states, W_pack, W_o, attention_mask)
    return out.astype(np.float32)



# revision 10
# speedup vs baseline: 1.1343x; 1.1343x over previous
"""Baichuan attention (B=2, S=1024, H=5120, NH=40, fp32) on 8 trn2 NeuronCores.

Strategy: tensor-parallel over heads (5 heads/core). Each core computes
qkv^T for its heads (fp16 matmuls, fp32 PSUM accumulate), causal+alibi
attention without max-subtraction (exp args are small; probs scaled by
1/64 to stay in fp16 range), and a partial o_proj over its 640
contraction dims. The 8 partial outputs are summed on the host.

The alibi mask is never shipped: slopes are derived from the mask input
on the host (mask[h, q, k] = causal + slope_h * k) and turned into
per-partition bias vectors for the exp activation; causality is handled
by only computing k-tiles at or below the diagonal plus a triangular
-1e30 mask on the diagonal tile.

The PE runs only real matmuls: the per-q softmax stabilizer is a DVE
scalar_tensor_tensor (q-ramp times per-head slope added into PSUM), and
the softmax normalizer Z is accumulated tile-wise on DVE then reduced
across partitions on the otherwise-idle GpSimd engine, whose all-reduce
output is already partition-broadcast (so no PE broadcast matmul).

All device-side layouts put the matmul contraction dim on partitions:
  xt    [B, 128, KT, S]    x^T tiles  (partition = hidden dim within k-tile)
  wqkv  [3*HPC, 128, KT, 128]  W_pack^T strips per output m-tile
  wo    [HPC, 128, H]      W_o^T strips (partition = per-core contraction dim)
  out   [B*QT, 128, H]     partial output, fp16 (token tiles on partitions)
"""

import math
from contextlib import ExitStack
from dataclasses import dataclass

import numpy as np

import concourse.bass as bass
import concourse.bass_isa as bass_isa
import concourse.mybir as mybir
from concourse import bacc
import concourse.tile as tile
from concourse import masks
from concourse.bass_utils import run_bass_kernel_spmd

F16 = mybir.dt.float16
F32 = mybir.dt.float32
P = 128
NEG = -60000.0
SCALE = 1.0 / math.sqrt(128.0)
LN_PSCALE = math.log(64.0)  # probs scaled by 1/64 so fp16 never overflows


@dataclass(frozen=True)
class Cfg:
    B: int = 2
    S: int = 1024
    KT: int = 40  # contraction tiles; H = KT * 128
    HPC: int = 5  # heads per core
    n_cores: int = 8

    @property
    def H(self):
        return self.KT * P

    @property
    def QT(self):
        return self.S // P

    @property
    def MQKV(self):
        return 3 * self.HPC

    @property
    def NBLK(self):
        return self.S // 512

    @property
    def OC(self):
        return self.H // 512


FULL = Cfg()


def build_nc(cfg: Cfg) -> bass.Bass:
    nc = bacc.Bacc("TRN2", debug=False)
    B, S, KT, HPC, QT, MQKV = cfg.B, cfg.S, cfg.KT, cfg.HPC, cfg.QT, cfg.MQKV

    xt_d = nc.dram_tensor("xt", [B, P, KT, S], F16, kind="ExternalInput")
    ws_d = nc.dram_tensor("wqkv", [MQKV, P, KT, P], F16, kind="ExternalInput")
    wo_d = nc.dram_tensor("wo", [HPC, P, cfg.H], F16, kind="ExternalInput")
    bias_d = nc.dram_tensor("bias", [P, HPC * QT], F32, kind="ExternalInput")
    qrbc_d = nc.dram_tensor("qrbc", [P, S], F16, kind="ExternalInput")
    slcp_d = nc.dram_tensor("slcp", [P, HPC], F32, kind="ExternalInput")
    ident_d = nc.dram_tensor("ident", [P, P], F16, kind="ExternalInput")
    tri_d = nc.dram_tensor("tri", [P, P], F16, kind="ExternalInput")
    out_d = nc.dram_tensor("out", [B * QT, P, cfg.H], F16, kind="ExternalOutput")

    with ExitStack() as ctx:
        tc = ctx.enter_context(tile.TileContext(nc))
        ctx.enter_context(
            nc.allow_low_precision(
                reason="fp16 Z/partial-out staging is within tolerance; "
                "PSUM accumulation is fp32 in-bank"
            )
        )
        consts = ctx.enter_context(tc.tile_pool(name="consts", bufs=1))
        xt_pool = ctx.enter_context(tc.tile_pool(name="xt", bufs=1))
        wqkv_pool = ctx.enter_context(tc.tile_pool(name="wqkv", bufs=2))
        qkvt_pool = ctx.enter_context(tc.tile_pool(name="qkvt", bufs=2))
        v_pool = ctx.enter_context(tc.tile_pool(name="v", bufs=4))
        p_pool = ctx.enter_context(tc.tile_pool(name="p", bufs=4))
        attnt_pool = ctx.enter_context(tc.tile_pool(name="attnt", bufs=2))
        atn_pool = ctx.enter_context(tc.tile_pool(name="atn", bufs=3))
        zacc_pool = ctx.enter_context(tc.tile_pool(name="zacc", bufs=2))
        vt_pool = ctx.enter_context(tc.tile_pool(name="vt", bufs=1))
        wo_pool = ctx.enter_context(tc.tile_pool(name="wo", bufs=3 * HPC))
        out_pool = ctx.enter_context(tc.tile_pool(name="out", bufs=4))
        mm_pool = ctx.enter_context(tc.tile_pool(name="mm", bufs=3, space="PSUM"))
        sc_pool = ctx.enter_context(tc.tile_pool(name="sc", bufs=3, space="PSUM"))
        acc_pool = ctx.enter_context(tc.tile_pool(name="acc", bufs=2, space="PSUM"))

        # constants (ident + causal tri shipped from the host: avoids the
        # gpsimd memset/iota/affine_select chain serializing kernel start)
        ident = consts.tile([P, P], F16)
        nc.sync.dma_start(ident[:], ident_d[:])
        tri = consts.tile([P, P], F16)
        nc.sync.dma_start(tri[:], tri_d[:])
        bias_sb = consts.tile([P, HPC * QT], F32)
        nc.sync.dma_start(bias_sb[:], bias_d[:])
        qrbc_sb = consts.tile([P, S], F16)
        nc.sync.dma_start(qrbc_sb[:], qrbc_d[:])
        slcp_sb = consts.tile([P, HPC], F32)
        nc.sync.dma_start(slcp_sb[:], slcp_d[:])

        # PE warm-up: self-contained matmuls on the identity tile keep the
        # PE busy through the p-state ramp while input DMAs stream
        warm = mm_pool.tile([P, 512], F32, tag="mm", name="warm")
        for _ in range(100):
            nc.tensor.matmul(warm[:, :P], ident[:], ident[:], start=True, stop=True)

        # last k-tile index contributing to each 512-wide q block
        def i_last(blk):
            return min(QT - 1, (blk + 1) * 4 - 1)

        if KT >= 8:
            sizes = [1, 2, 4, 5]
            rem = KT - sum(sizes)
            nrem = 4
            q, r = divmod(rem, nrem)
            sizes += [q + (1 if i < r else 0) for i in range(nrem)]
        else:
            sizes = [1] * KT
        k2chunk = []
        for ci, s in enumerate(sizes):
            for j in range(s):
                k2chunk.append((ci, j))
        state = {}

        chunk_c0 = []
        c0 = 0
        for s in sizes:
            chunk_c0.append(c0)
            c0 += s
        # k index after whose matmuls chunk ci has no further readers in
        # this m-tile (used to prefetch the next batch's chunk)
        chunk_last_k = [c + s - 1 for c, s in zip(chunk_c0, sizes)]

        def load_xt_chunk(b, ci):
            s = sizes[ci]
            xc = xt_pool.tile([P, s, S], F16, tag=f"xt{ci}", name=f"xt{ci}")
            nc.sync.dma_start(xc[:], xt_d[b, :, chunk_c0[ci] : chunk_c0[ci] + s, :])
            state.setdefault((b, "xtc"), {})[ci] = xc
            if len(state[b, "xtc"]) == len(sizes):
                state[b, "xt"] = [state[b, "xtc"][i] for i in range(len(sizes))]

        def load_xt(b):
            # chunk tiles with progressive sizes: QKV starts as soon as the
            # first small chunk lands instead of after the full 10MB
            for ci in range(len(sizes)):
                load_xt_chunk(b, ci)

        def prefetch_ws(b, m):
            ws = wqkv_pool.tile([P, KT, P], F16, tag="ws", name=f"ws{b}_{m}")
            nc.sync.dma_start(ws[:], ws_d[m])
            state[b, "ws", m] = ws

        def qkv_mtile(b, m, chunk_hook=None):
            # one 128-row strip of qkv^T = W^T.T @ x^T (contraction over H)
            if (b, "qkvt") not in state:
                state[b, "qkvt"] = qkvt_pool.tile(
                    [P, 2 * HPC, S], F16, tag="qkvt", name=f"qkvt{b}"
                )
            qkvt_sb = state[b, "qkvt"]
            xt_ch = state[b, "xt"]
            if (b, "ws", m) in state:
                ws = state.pop((b, "ws", m))
            else:
                ws = wqkv_pool.tile([P, KT, P], F16, tag="ws", name=f"ws{b}_{m}")
                nc.sync.dma_start(ws[:], ws_d[m])
            ps = [
                mm_pool.tile([P, 512], F32, tag="mm", name=f"ps{hf}")
                for hf in range(S // 512)
            ]
            for k in range(KT):
                for hf in range(S // 512):
                    nc.tensor.matmul(
                        ps[hf][:],
                        ws[:, k, :],
                        xt_ch[k2chunk[k][0]][:, k2chunk[k][1], hf * 512 : (hf + 1) * 512],
                        start=(k == 0),
                        stop=(k == KT - 1),
                    )
                if chunk_hook is not None and k == chunk_last_k[k2chunk[k][0]]:
                    chunk_hook(k2chunk[k][0])
                if b == 0 and m == 0 and k == chunk_last_k[k2chunk[k][0]] and k < KT - 1:
                    # first pass races the x^T DMA: keep the PE warm across
                    # the chunk-arrival gaps (sc pool is idle until attention)
                    fl = sc_pool.tile([P, 512], F32, tag="sc", name="fill")
                    for _ in range(10):
                        nc.tensor.matmul(
                            fl[:, :P], ident[:], ident[:], start=True, stop=True
                        )
                if k % 3 == 2:
                    yield
            if m < 2 * HPC:
                # PSUM->SBUF staging on the Scalar engine (Copy shares the
                # act table with Exp): keeps the DVE free for attention work
                for hf in range(S // 512):
                    nc.scalar.activation(
                        qkvt_sb[:, m, hf * 512 : (hf + 1) * 512],
                        ps[hf][:],
                        mybir.ActivationFunctionType.Copy,
                    )
            else:
                # v^T strip: stage, then PE-transpose to per-head natural V
                hh = m - 2 * HPC
                vt = vt_pool.tile([P, S], F16, tag="vt", name=f"vt{b}_{hh}")
                for hf in range(S // 512):
                    nc.scalar.activation(
                        vt[:, hf * 512 : (hf + 1) * 512],
                        ps[hf][:],
                        mybir.ActivationFunctionType.Copy,
                    )
                v_sb = v_pool.tile([P, QT, P], F16, tag="v", name=f"v{b}_{hh}")
                state[b, "v", hh] = v_sb
                for i in range(QT):
                    tp = mm_pool.tile([P, P], F16, tag="mm")
                    nc.tensor.transpose(tp[:], vt[:, i * P : (i + 1) * P], ident[:])
                    nc.vector.tensor_copy(v_sb[:, i, :], tp[:])
                    if i % 4 == 3:
                        yield

        def attn_head(b, hh, zacc_ref, zeng=None):
            # scores^T = K^T.T @ Q^T with k-positions on partitions; causal
            # ragged tiles; p = exp(s/sqrt(d) + alibi_k - slope*q - ln64)
            if (b, "attnt") not in state:
                state[b, "attnt"] = attnt_pool.tile(
                    [P, HPC, S], F16, tag="attnt", name=f"attnt{b}"
                )
            attnt_sb = state[b, "attnt"]
            qkvt_sb = state[b, "qkvt"]
            v_sb = state[b, "v", hh]
            at = [
                acc_pool.tile([P, 512], F32, tag="acc", name=f"at{blk}")
                for blk in range(cfg.NBLK)
            ]
            zacc = [
                zacc_pool.tile([P, 512], F32, tag=f"zacc{blk}", name=f"z{b}_{hh}_{blk}")
                for blk in range(cfg.NBLK)
            ]
            zacc_ref["zacc"] = zacc

            def pv_stage(i, blk, off, w, pt):
                # Z partial sums: tile-wise accumulate (partition reduction
                # happens once per block on GpSimd at the end)
                if i == 0:
                    zeng.tensor_copy(zacc[blk][:, off : off + w], pt[:, :w])
                else:
                    zeng.tensor_tensor(
                        zacc[blk][:, off : off + w],
                        zacc[blk][:, off : off + w],
                        pt[:, :w],
                        mybir.AluOpType.add,
                    )
                nc.tensor.matmul(
                    at[blk][:, off : off + w],
                    v_sb[:, i, :],
                    pt[:, :w],
                    start=(i == 0),
                    stop=(i == i_last(blk)),
                )
                if i == i_last(blk):
                    # evacuate finished PV accumulator to SBUF right away so
                    # the PSUM bank frees for the next head without waiting
                    # on the Z reduction chain, and kick off the Z partition
                    # reduction on GpSimd immediately so its ~2.5us latency
                    # hides under the remaining tiles
                    atn = atn_pool.tile(
                        [P, 512], F16, tag="atn", name=f"atn{b}_{hh}_{blk}"
                    )
                    state[b, "atn", hh, blk] = atn
                    nc.vector.tensor_copy(atn[:], at[blk][:])
                    nc.gpsimd.partition_all_reduce(
                        zacc[blk][:],
                        zacc[blk][:],
                        channels=P,
                        reduce_op=bass_isa.ReduceOp.add,
                    )

            # software-pipelined by two tiles: tile n's PV matmul is emitted
            # after tile n+2's score matmul, so the PE's in-order queue never
            # parks on the stabilizer->exp chain — in particular right after
            # a head boundary, where the chain restarts with no backlog
            pend = []
            for i in range(QT):
                k0 = i * P
                for blk in range(cfg.NBLK):
                    c0 = max(blk * 512, k0)
                    c1 = (blk + 1) * 512
                    if c0 >= c1:
                        continue  # q block entirely above the diagonal
                    w = c1 - c0
                    sc = sc_pool.tile([P, 512], F32, tag="sc")
                    nc.tensor.matmul(
                        sc[:, :w],
                        qkvt_sb[:, HPC + hh, k0 : k0 + P],
                        qkvt_sb[:, hh, c0:c1],
                        start=True,
                        stop=True,
                    )
                    # per-q stabilizer: scores += -slope*q*sqrt(d) (any per-q
                    # shift cancels in the softmax normalization); DVE fused
                    # multiply-add keeps this off the PE
                    nc.vector.scalar_tensor_tensor(
                        out=sc[:, :w],
                        in0=qrbc_sb[:, c0:c1],
                        scalar=slcp_sb[:, hh : hh + 1],
                        in1=sc[:, :w],
                        op0=mybir.AluOpType.mult,
                        op1=mybir.AluOpType.add,
                    )
                    if c0 == k0:  # diagonal tile: causal triangle
                        nc.vector.tensor_tensor(
                            sc[:, :P], sc[:, :P], tri[:], mybir.AluOpType.add
                        )
                    pt = p_pool.tile([P, 512], F16, tag="p")
                    nc.scalar.activation(
                        pt[:, :w],
                        sc[:, :w],
                        mybir.ActivationFunctionType.Exp,
                        bias=bias_sb[:, hh * QT + i : hh * QT + i + 1],
                        scale=SCALE,
                    )
                    if len(pend) >= 2:
                        pv_stage(*pend.pop(0))
                    pend.append((i, blk, c0 - blk * 512, w, pt))
                    yield
            # leave the final tiles' PVs to the caller: flushing them after
            # the NEXT head's first score matmuls gives their exp chains
            # breathing room instead of stalling the PE at the head boundary
            zacc_ref["flush"] = lambda: pv_stage(*pend.pop(0)) if pend else None

        def attn_tail_a(b, hh, zacc):
            # 1/Z: reciprocal of a single partition row (DVE reciprocal on a
            # full 128x512 tile costs ~4us and head-of-line blocks the DVE
            # queue), then partition-broadcast it on the idle GpSimd
            for blk in range(cfg.NBLK):
                nc.vector.reciprocal_approx_fast(
                    out=zacc[blk][0:1, :], in_=zacc[blk][0:1, :]
                )
            for blk in range(cfg.NBLK):
                nc.gpsimd.partition_broadcast(
                    zacc[blk][:], zacc[blk][0:1, :], channels=P
                )

        def attn_tail_b(b, hh, zacc):
            # normalize: attnT = attnT_unnorm * (1/Z); pure SBUF DVE multiply
            attnt_sb = state[b, "attnt"]
            for blk in range(cfg.NBLK):
                nc.vector.tensor_tensor(
                    attnt_sb[:, hh, blk * 512 : (blk + 1) * 512],
                    state.pop((b, "atn", hh, blk))[:],
                    zacc[blk][:],
                    mybir.AluOpType.mult,
                )

        def attn_stream(b, zeng):
            # run the 5 heads back to back; each head's normalize chain runs
            # deferred in two stages during the NEXT head so neither the
            # GpSimd partition reduction nor the broadcast ever stalls the
            # DVE queue that feeds the stabilizer->exp->PV pipeline
            pending = None
            flush = None
            for hh in range(HPC):
                zacc_ref = {}
                g = attn_head(b, hh, zacc_ref, zeng)
                for j, _ in enumerate(g):
                    yield
                    if j in (0, 1) and flush is not None:
                        flush()
                        if j == 1:
                            flush = None
                    if pending is not None:
                        if j == 5:
                            attn_tail_a(*pending)
                        elif j == 9:
                            attn_tail_b(*pending)
                            pending = None
                pending = (b, hh, zacc_ref["zacc"])
                flush = zacc_ref["flush"]
            if flush is not None:
                flush()
                flush()
            if pending is not None:
                attn_tail_a(*pending)
                attn_tail_b(*pending)

        wo_cache = {}  # oc -> wos; at most 3 chunks resident (bufs = 3*HPC)

        def load_wo(oc):
            # W_o strips are batch-independent; cache by chunk so the second
            # batch's o_proj can reuse whatever is still resident
            if oc in wo_cache:
                return wo_cache[oc]
            while len(wo_cache) >= 3:
                # the pool recycles the oldest buffers; drop its cache entry
                wo_cache.pop(next(iter(wo_cache)))
            wos = []
            for k in range(HPC):
                wt = wo_pool.tile([P, 512], F16, tag="wo", name=f"wo{oc}_{k}")
                nc.sync.dma_start(wt[:], wo_d[k, :, oc * 512 : (oc + 1) * 512])
                wos.append(wt)
            wo_cache[oc] = wos
            return wos

        def oproj_chunk(b, oc, prefetch=None, pools=None):
            # out[t, oc] partial: contraction over this core's 5*128 dims
            attnt_sb = state[b, "attnt"]
            wos = load_wo(oc)
            pools = pools or [(mm_pool, "mm")]
            for t in range(QT):
                if t == 1 and prefetch is not None:
                    load_wo(prefetch)
                pool, ptag = pools[t % len(pools)]
                po = pool.tile([P, 512], F32, tag=ptag)
                for k in range(HPC):
                    nc.tensor.matmul(
                        po[:],
                        attnt_sb[:, k, t * P : (t + 1) * P],
                        wos[k][:],
                        start=(k == 0),
                        stop=(k == HPC - 1),
                    )
                # stage to fp16 on the Scalar engine (Copy shares the act
                # table with Exp, so no table reloads) to keep the DVE free
                ot = out_pool.tile([P, 512], F16, tag="ot")
                nc.scalar.activation(
                    ot[:], po[:], mybir.ActivationFunctionType.Copy
                )
                nc.sync.dma_start(
                    out_d[b * QT + t, :, oc * 512 : (oc + 1) * 512], ot[:]
                )
                yield

        def drain(gens):
            for g in gens:
                for _ in g:
                    pass

        def interleave(a_gens, b_gens, ratio):
            """Step generator stream a, inserting `ratio` (possibly
            fractional) steps of stream b after each a-step. Instruction-level
            pipelining: b's big dense matmuls fill a's dependency stalls so
            the PE never idles long enough for HAM to re-throttle. A
            fractional ratio paces b evenly across ALL of a instead of
            exhausting it early and leaving a's tail stalls uncovered."""
            bi = 0
            credit = 0.0
            for g in a_gens:
                for _ in g:
                    credit += ratio
                    while credit >= 1.0 and bi < len(b_gens):
                        try:
                            next(b_gens[bi])
                            credit -= 1.0
                        except StopIteration:
                            bi += 1
            drain(b_gens[bi:])

        # ---- software pipeline: keep the PE stream dense so HAM stays warm
        prefetch_ws(0, 0)
        load_xt(0)
        drain([qkv_mtile(0, m) for m in range(MQKV)])
        load_xt(1)
        # fill-to-attention yield ratios pace the fills across the WHOLE
        # attention stream: qkv(1) has 205 yields / 60 attn yields; oproj(0)
        # has 80 yields / 60 attn yields
        interleave(
            [attn_stream(0, nc.vector)],
            [qkv_mtile(1, m) for m in range(MQKV)],
            ratio=3.4,
        )
        interleave(
            [attn_stream(1, nc.vector)],
            [
                oproj_chunk(0, oc, prefetch=(oc + 1) if oc + 1 < cfg.OC else 0)
                for oc in range(cfg.OC)
            ],
            ratio=1.32,
        )
        drain(
            [
                oproj_chunk(
                    1,
                    oc,
                    prefetch=(oc + 1) if oc + 1 < cfg.OC else None,
                    pools=[(mm_pool, "mm"), (sc_pool, "sc")],
                )
                for oc in range(cfg.OC)
            ]
        )

    nc.compile()
    return nc


def prep_inputs(hidden_states, W_pack, W_o, attention_mask, cfg: Cfg = FULL):
    """Shard + lay out the full inputs for the 8 cores. Returns in_maps."""
    B, S, KT, HPC = cfg.B, cfg.S, cfg.KT, cfg.HPC
    H = cfg.H
    hs = np.asarray(hidden_states)
    wp = np.asarray(W_pack)
    wo = np.asarray(W_o)
    am = np.asarray(attention_mask)

    # x^T layout [B, 128, KT, S]: xt[b, p, k, t] = hs[b, t, k*128 + p]
    xt = np.ascontiguousarray(
        hs.reshape(B, S, KT, P).transpose(0, 3, 2, 1).astype(np.float16)
    )

    # alibi slopes from the mask: mask[h, q, k] = causal + slope_h * k
    slopes = am[:, -1, 1].astype(np.float64)  # mask[h, S-1, 1] = slope_h

    # q-position ramp, identical in every partition (exact in fp16)
    qrbc = np.tile(np.arange(S, dtype=np.float16)[None, :], (P, 1))

    # identity + causal triangle (tri[k, q] = NEG where k > q), host-built
    ident = np.eye(P, dtype=np.float16)
    tri = np.where(
        np.arange(P)[:, None] > np.arange(P)[None, :], np.float16(NEG), np.float16(0)
    ).astype(np.float16)

    kvec = np.arange(P, dtype=np.float64)
    in_maps = []
    for c in range(cfg.n_cores):
        heads = range(c * HPC, (c + 1) * HPC)
        # W_pack^T strips: m-tiles [q0..q4, k0..k4, v0..v4] for this core's heads
        rows = []
        for sec in range(3):  # q, k, v blocks of W_pack
            for h in heads:
                r0 = sec * H + h * P
                rows.append(wp[r0 : r0 + P, :])  # [128, H]
        # strip[m, p, k, j] = W_pack[row_j, k*128 + p]
        ws = np.stack(
            [r.T.reshape(KT, P, P).transpose(1, 0, 2) for r in rows]
        ).astype(np.float16)

        # W_o^T strip: wo_c[k, p, o] = W_o[o, c*HPC*128 + k*128 + p]
        wo_c = np.ascontiguousarray(
            wo[:, c * HPC * P : (c + 1) * HPC * P].T.reshape(HPC, P, H)
        ).astype(np.float16)

        # exp bias table [128, HPC*QT]: col hh*QT + i -> slope*(i*128+k) - lnPS
        bias = np.empty((P, HPC * cfg.QT), dtype=np.float32)
        # stabilizer slope column: -slope_h * sqrt(d), replicated down partitions
        slcp = np.empty((P, HPC), dtype=np.float32)
        for hh, h in enumerate(heads):
            for i in range(cfg.QT):
                bias[:, hh * cfg.QT + i] = (
                    slopes[h] * (i * P + kvec) - LN_PSCALE
                ).astype(np.float32)
            slcp[:, hh] = np.float32(-slopes[h] * math.sqrt(128.0))

        in_maps.append(
            {
                "xt": xt,
                "wqkv": np.ascontiguousarray(ws),
                "wo": wo_c,
                "bias": bias,
                "qrbc": qrbc,
                "slcp": slcp,
                "ident": ident,
                "tri": tri,
            }
        )
    return in_maps


_CACHE = {}


def _get_nc(cfg: Cfg = FULL) -> bass.Bass:
    if cfg not in _CACHE:
        _CACHE[cfg] = build_nc(cfg)
    return _CACHE[cfg]


def run(hidden_states, W_pack, W_o, attention_mask, cfg: Cfg = FULL, **kw):
    nc = _get_nc(cfg)
    in_maps = prep_inputs(hidden_states, W_pack, W_o, attention_mask, cfg)
    res = run_bass_kernel_spmd(nc, in_maps, core_ids=list(range(cfg.n_cores)), **kw)
    # sum the per-core partials (fp16 -> fp32), unshard to [B, S, H]
    acc = np.zeros((cfg.B * cfg.QT, P, cfg.H), dtype=np.float32)
    for r in res.results:
        acc += r["out"].astype(np.float32)
    out = acc.reshape(cfg.B, cfg.S, cfg.H)
    return out, res


def kernel(hidden_states, W_pack, W_o, attention_mask):
    out, _ = run(hidden_states, W_pack, W_o, attention_mask)
    return out.astype(np.float32)



# revision 11
# speedup vs baseline: 1.1555x; 1.0187x over previous
"""Baichuan attention (B=2, S=1024, H=5120, NH=40, fp32) on 8 trn2 NeuronCores.

Strategy: tensor-parallel over heads (5 heads/core). Each core computes
qkv^T for its heads (fp16 matmuls, fp32 PSUM accumulate), causal+alibi
attention without max-subtraction (exp args are small; probs scaled by
1/64 to stay in fp16 range), and a partial o_proj over its 640
contraction dims. The 8 partial outputs are summed on the host.

The alibi mask is never shipped: slopes are derived from the mask input
on the host (mask[h, q, k] = causal + slope_h * k) and turned into
per-partition bias vectors for the exp activation; causality is handled
by only computing k-tiles at or below the diagonal plus a triangular
-1e30 mask on the diagonal tile.

The PE runs only real matmuls: the per-q softmax stabilizer is a DVE
scalar_tensor_tensor (q-ramp times per-head slope added into PSUM), and
the softmax normalizer Z is accumulated tile-wise on DVE then reduced
across partitions on the otherwise-idle GpSimd engine, whose all-reduce
output is already partition-broadcast (so no PE broadcast matmul).

All device-side layouts put the matmul contraction dim on partitions:
  xt    [B, 128, KT, S]    x^T tiles  (partition = hidden dim within k-tile)
  wqkv  [3*HPC, 128, KT, 128]  W_pack^T strips per output m-tile
  wo    [HPC, 128, H]      W_o^T strips (partition = per-core contraction dim)
  out   [B*QT, 128, H]     partial output, fp16 (token tiles on partitions)
"""

import math
from contextlib import ExitStack
from dataclasses import dataclass

import numpy as np

import concourse.bass as bass
import concourse.bass_isa as bass_isa
import concourse.mybir as mybir
from concourse import bacc
import concourse.tile as tile
from concourse import masks
from concourse.bass_utils import run_bass_kernel_spmd

F16 = mybir.dt.float16
F32 = mybir.dt.float32
P = 128
NEG = -60000.0
SCALE = 1.0 / math.sqrt(128.0)
LN_PSCALE = math.log(64.0)  # probs scaled by 1/64 so fp16 never overflows


@dataclass(frozen=True)
class Cfg:
    B: int = 2
    S: int = 1024
    KT: int = 40  # contraction tiles; H = KT * 128
    HPC: int = 5  # heads per core
    n_cores: int = 8

    @property
    def H(self):
        return self.KT * P

    @property
    def QT(self):
        return self.S // P

    @property
    def MQKV(self):
        return 3 * self.HPC

    @property
    def NBLK(self):
        return self.S // 512

    @property
    def OC(self):
        return self.H // 512


FULL = Cfg()


def build_nc(cfg: Cfg) -> bass.Bass:
    nc = bacc.Bacc("TRN2", debug=False)
    B, S, KT, HPC, QT, MQKV = cfg.B, cfg.S, cfg.KT, cfg.HPC, cfg.QT, cfg.MQKV

    xt_d = nc.dram_tensor("xt", [B, P, KT, S], F16, kind="ExternalInput")
    ws_d = nc.dram_tensor("wqkv", [MQKV, P, KT, P], F16, kind="ExternalInput")
    wo_d = nc.dram_tensor("wo", [HPC, P, cfg.H], F16, kind="ExternalInput")
    bias_d = nc.dram_tensor("bias", [P, HPC * QT], F32, kind="ExternalInput")
    qrbc_d = nc.dram_tensor("qrbc", [P, S], F16, kind="ExternalInput")
    slcp_d = nc.dram_tensor("slcp", [P, HPC], F32, kind="ExternalInput")
    out_d = nc.dram_tensor("out", [B * QT, P, cfg.H], F16, kind="ExternalOutput")

    with ExitStack() as ctx:
        tc = ctx.enter_context(tile.TileContext(nc))
        ctx.enter_context(
            nc.allow_low_precision(
                reason="fp16 Z/partial-out staging is within tolerance; "
                "PSUM accumulation is fp32 in-bank"
            )
        )
        consts = ctx.enter_context(tc.tile_pool(name="consts", bufs=1))
        xt_pool = ctx.enter_context(tc.tile_pool(name="xt", bufs=1))
        wqkv_pool = ctx.enter_context(tc.tile_pool(name="wqkv", bufs=2))
        qkvt_pool = ctx.enter_context(tc.tile_pool(name="qkvt", bufs=2))
        v_pool = ctx.enter_context(tc.tile_pool(name="v", bufs=4))
        p_pool = ctx.enter_context(tc.tile_pool(name="p", bufs=4))
        attnt_pool = ctx.enter_context(tc.tile_pool(name="attnt", bufs=2))
        atn_pool = ctx.enter_context(tc.tile_pool(name="atn", bufs=3))
        zacc_pool = ctx.enter_context(tc.tile_pool(name="zacc", bufs=2))
        vt_pool = ctx.enter_context(tc.tile_pool(name="vt", bufs=1))
        wo_pool = ctx.enter_context(tc.tile_pool(name="wo", bufs=3 * HPC))
        out_pool = ctx.enter_context(tc.tile_pool(name="out", bufs=4))
        mm_pool = ctx.enter_context(tc.tile_pool(name="mm", bufs=3, space="PSUM"))
        sc_pool = ctx.enter_context(tc.tile_pool(name="sc", bufs=3, space="PSUM"))
        acc_pool = ctx.enter_context(tc.tile_pool(name="acc", bufs=2, space="PSUM"))

        # constants
        ident = consts.tile([P, P], F16)
        masks.make_identity(nc, ident[:])
        tri = consts.tile([P, P], F16)
        # tri[k, q] = NEG where k > q (strictly below diagonal), else 0
        nc.gpsimd.memset(tri[:], 0.0)
        nc.gpsimd.affine_select(
            out=tri[:],
            in_=tri[:],
            compare_op=mybir.AluOpType.is_ge,
            fill=NEG,
            base=0,
            # keep where (q - k) >= 0, fill NEG where k > q
            pattern=[[1, P]],
            channel_multiplier=-1,
        )
        bias_sb = consts.tile([P, HPC * QT], F32)
        nc.sync.dma_start(bias_sb[:], bias_d[:])
        qrbc_sb = consts.tile([P, S], F16)
        nc.sync.dma_start(qrbc_sb[:], qrbc_d[:])
        slcp_sb = consts.tile([P, HPC], F32)
        nc.sync.dma_start(slcp_sb[:], slcp_d[:])

        # PE warm-up: self-contained matmuls on the identity tile keep the
        # PE busy through the p-state ramp while input DMAs stream
        warm = mm_pool.tile([P, 512], F32, tag="mm", name="warm")
        for _ in range(185):
            nc.tensor.matmul(warm[:, :P], ident[:], ident[:], start=True, stop=True)

        # last k-tile index contributing to each 512-wide q block
        def i_last(blk):
            return min(QT - 1, (blk + 1) * 4 - 1)

        if KT >= 8:
            sizes = [1, 2, 4, 5]
            rem = KT - sum(sizes)
            nrem = 4
            q, r = divmod(rem, nrem)
            sizes += [q + (1 if i < r else 0) for i in range(nrem)]
        else:
            sizes = [1] * KT
        k2chunk = []
        for ci, s in enumerate(sizes):
            for j in range(s):
                k2chunk.append((ci, j))
        state = {}

        chunk_c0 = []
        c0 = 0
        for s in sizes:
            chunk_c0.append(c0)
            c0 += s
        # k index after whose matmuls chunk ci has no further readers in
        # this m-tile (used to prefetch the next batch's chunk)
        chunk_last_k = [c + s - 1 for c, s in zip(chunk_c0, sizes)]

        def load_xt_chunk(b, ci):
            s = sizes[ci]
            xc = xt_pool.tile([P, s, S], F16, tag=f"xt{ci}", name=f"xt{ci}")
            nc.sync.dma_start(xc[:], xt_d[b, :, chunk_c0[ci] : chunk_c0[ci] + s, :])
            state.setdefault((b, "xtc"), {})[ci] = xc
            if len(state[b, "xtc"]) == len(sizes):
                state[b, "xt"] = [state[b, "xtc"][i] for i in range(len(sizes))]

        def load_xt(b):
            # chunk tiles with progressive sizes: QKV starts as soon as the
            # first small chunk lands instead of after the full 10MB
            for ci in range(len(sizes)):
                load_xt_chunk(b, ci)

        def prefetch_ws(b, m):
            ws = wqkv_pool.tile([P, KT, P], F16, tag="ws", name=f"ws{b}_{m}")
            nc.sync.dma_start(ws[:], ws_d[m])
            state[b, "ws", m] = ws

        def qkv_mtile(b, m, chunk_hook=None):
            # one 128-row strip of qkv^T = W^T.T @ x^T (contraction over H)
            if (b, "qkvt") not in state:
                state[b, "qkvt"] = qkvt_pool.tile(
                    [P, 2 * HPC, S], F16, tag="qkvt", name=f"qkvt{b}"
                )
            qkvt_sb = state[b, "qkvt"]
            xt_ch = state[b, "xt"]
            if (b, "ws", m) in state:
                ws = state.pop((b, "ws", m))
            else:
                ws = wqkv_pool.tile([P, KT, P], F16, tag="ws", name=f"ws{b}_{m}")
                nc.sync.dma_start(ws[:], ws_d[m])
            ps = [
                mm_pool.tile([P, 512], F32, tag="mm", name=f"ps{hf}")
                for hf in range(S // 512)
            ]
            for k in range(KT):
                for hf in range(S // 512):
                    nc.tensor.matmul(
                        ps[hf][:],
                        ws[:, k, :],
                        xt_ch[k2chunk[k][0]][:, k2chunk[k][1], hf * 512 : (hf + 1) * 512],
                        start=(k == 0),
                        stop=(k == KT - 1),
                    )
                if chunk_hook is not None and k == chunk_last_k[k2chunk[k][0]]:
                    chunk_hook(k2chunk[k][0])
                if b == 0 and m == 0 and k == chunk_last_k[k2chunk[k][0]] and k < KT - 1:
                    # first pass races the x^T DMA: keep the PE warm across
                    # the chunk-arrival gaps (sc pool is idle until attention)
                    fl = sc_pool.tile([P, 512], F32, tag="sc", name="fill")
                    for _ in range(10):
                        nc.tensor.matmul(
                            fl[:, :P], ident[:], ident[:], start=True, stop=True
                        )
                if k % 3 == 2:
                    yield
            if m < 2 * HPC:
                # PSUM->SBUF staging on the Scalar engine (Copy shares the
                # act table with Exp): keeps the DVE free for attention work
                for hf in range(S // 512):
                    nc.scalar.activation(
                        qkvt_sb[:, m, hf * 512 : (hf + 1) * 512],
                        ps[hf][:],
                        mybir.ActivationFunctionType.Copy,
                    )
            else:
                # v^T strip: stage, then PE-transpose to per-head natural V
                hh = m - 2 * HPC
                vt = vt_pool.tile([P, S], F16, tag="vt", name=f"vt{b}_{hh}")
                for hf in range(S // 512):
                    nc.scalar.activation(
                        vt[:, hf * 512 : (hf + 1) * 512],
                        ps[hf][:],
                        mybir.ActivationFunctionType.Copy,
                    )
                v_sb = v_pool.tile([P, QT, P], F16, tag="v", name=f"v{b}_{hh}")
                state[b, "v", hh] = v_sb
                for i in range(QT):
                    tp = mm_pool.tile([P, P], F16, tag="mm")
                    nc.tensor.transpose(tp[:], vt[:, i * P : (i + 1) * P], ident[:])
                    nc.vector.tensor_copy(v_sb[:, i, :], tp[:])
                    if i % 4 == 3:
                        yield

        def attn_head(b, hh, zacc_ref, zeng=None):
            # scores^T = K^T.T @ Q^T with k-positions on partitions; causal
            # ragged tiles; p = exp(s/sqrt(d) + alibi_k - slope*q - ln64)
            if (b, "attnt") not in state:
                state[b, "attnt"] = attnt_pool.tile(
                    [P, HPC, S], F16, tag="attnt", name=f"attnt{b}"
                )
            attnt_sb = state[b, "attnt"]
            qkvt_sb = state[b, "qkvt"]
            v_sb = state[b, "v", hh]
            at = [
                acc_pool.tile([P, 512], F32, tag="acc", name=f"at{blk}")
                for blk in range(cfg.NBLK)
            ]
            zacc = [
                zacc_pool.tile([P, 512], F32, tag=f"zacc{blk}", name=f"z{b}_{hh}_{blk}")
                for blk in range(cfg.NBLK)
            ]
            zacc_ref["zacc"] = zacc

            def pv_stage(i, blk, off, w, pt):
                # Z partial sums: tile-wise accumulate (partition reduction
                # happens once per block on GpSimd at the end)
                if i == 0:
                    zeng.tensor_copy(zacc[blk][:, off : off + w], pt[:, :w])
                else:
                    zeng.tensor_tensor(
                        zacc[blk][:, off : off + w],
                        zacc[blk][:, off : off + w],
                        pt[:, :w],
                        mybir.AluOpType.add,
                    )
                nc.tensor.matmul(
                    at[blk][:, off : off + w],
                    v_sb[:, i, :],
                    pt[:, :w],
                    start=(i == 0),
                    stop=(i == i_last(blk)),
                )
                if i == i_last(blk):
                    # evacuate finished PV accumulator to SBUF right away so
                    # the PSUM bank frees for the next head without waiting
                    # on the Z reduction chain, and kick off the Z partition
                    # reduction on GpSimd immediately so its ~2.5us latency
                    # hides under the remaining tiles
                    atn = atn_pool.tile(
                        [P, 512], F16, tag="atn", name=f"atn{b}_{hh}_{blk}"
                    )
                    state[b, "atn", hh, blk] = atn
                    nc.vector.tensor_copy(atn[:], at[blk][:])
                    nc.gpsimd.partition_all_reduce(
                        zacc[blk][:],
                        zacc[blk][:],
                        channels=P,
                        reduce_op=bass_isa.ReduceOp.add,
                    )

            # software-pipelined by one tile: tile n's PV matmul is emitted
            # after tile n+1's score matmul, so the PE's in-order queue never
            # parks on the stabilizer->exp chain before filler can run
            pv_prev = None
            for i in range(QT):
                k0 = i * P
                for blk in range(cfg.NBLK):
                    c0 = max(blk * 512, k0)
                    c1 = (blk + 1) * 512
                    if c0 >= c1:
                        continue  # q block entirely above the diagonal
                    w = c1 - c0
                    sc = sc_pool.tile([P, 512], F32, tag="sc")
                    nc.tensor.matmul(
                        sc[:, :w],
                        qkvt_sb[:, HPC + hh, k0 : k0 + P],
                        qkvt_sb[:, hh, c0:c1],
                        start=True,
                        stop=True,
                    )
                    # per-q stabilizer: scores += -slope*q*sqrt(d) (any per-q
                    # shift cancels in the softmax normalization); DVE fused
                    # multiply-add keeps this off the PE
                    nc.vector.scalar_tensor_tensor(
                        out=sc[:, :w],
                        in0=qrbc_sb[:, c0:c1],
                        scalar=slcp_sb[:, hh : hh + 1],
                        in1=sc[:, :w],
                        op0=mybir.AluOpType.mult,
                        op1=mybir.AluOpType.add,
                    )
                    if c0 == k0:  # diagonal tile: causal triangle
                        nc.vector.tensor_tensor(
                            sc[:, :P], sc[:, :P], tri[:], mybir.AluOpType.add
                        )
                    pt = p_pool.tile([P, 512], F16, tag="p")
                    nc.scalar.activation(
                        pt[:, :w],
                        sc[:, :w],
                        mybir.ActivationFunctionType.Exp,
                        bias=bias_sb[:, hh * QT + i : hh * QT + i + 1],
                        scale=SCALE,
                    )
                    if pv_prev is not None:
                        pv_stage(*pv_prev)
                    pv_prev = (i, blk, c0 - blk * 512, w, pt)
                    yield
            # leave the final tile's PV to the caller: flushing it after the
            # NEXT head's first score matmul gives its exp chain ~2.5us of
            # breathing room instead of stalling the PE at the head boundary
            zacc_ref["flush"] = lambda: pv_stage(*pv_prev)

        def attn_tail_a(b, hh, zacc):
            # 1/Z: reciprocal of a single partition row (DVE reciprocal on a
            # full 128x512 tile costs ~4us and head-of-line blocks the DVE
            # queue), then partition-broadcast it on the idle GpSimd
            for blk in range(cfg.NBLK):
                nc.vector.reciprocal_approx_fast(
                    out=zacc[blk][0:1, :], in_=zacc[blk][0:1, :]
                )
            for blk in range(cfg.NBLK):
                nc.gpsimd.partition_broadcast(
                    zacc[blk][:], zacc[blk][0:1, :], channels=P
                )

        def attn_tail_b(b, hh, zacc):
            # normalize: attnT = attnT_unnorm * (1/Z); pure SBUF DVE multiply
            attnt_sb = state[b, "attnt"]
            for blk in range(cfg.NBLK):
                nc.vector.tensor_tensor(
                    attnt_sb[:, hh, blk * 512 : (blk + 1) * 512],
                    state.pop((b, "atn", hh, blk))[:],
                    zacc[blk][:],
                    mybir.AluOpType.mult,
                )

        def attn_stream(b, zeng):
            # run the 5 heads back to back; each head's normalize chain runs
            # deferred in two stages during the NEXT head so neither the
            # GpSimd partition reduction nor the broadcast ever stalls the
            # DVE queue that feeds the stabilizer->exp->PV pipeline
            pending = None
            flush = None
            for hh in range(HPC):
                zacc_ref = {}
                g = attn_head(b, hh, zacc_ref, zeng)
                for j, _ in enumerate(g):
                    yield
                    if j == 0 and flush is not None:
                        flush()
                        flush = None
                    if pending is not None:
                        if j == 4:
                            attn_tail_a(*pending)
                        elif j == 8:
                            attn_tail_b(*pending)
                            pending = None
                pending = (b, hh, zacc_ref["zacc"])
                flush = zacc_ref["flush"]
            if flush is not None:
                flush()
            if pending is not None:
                attn_tail_a(*pending)
                attn_tail_b(*pending)

        wo_cache = {}  # oc -> wos; at most 3 chunks resident (bufs = 3*HPC)

        def load_wo(oc):
            # W_o strips are batch-independent; cache by chunk so the second
            # batch's o_proj can reuse whatever is still resident
            if oc in wo_cache:
                return wo_cache[oc]
            while len(wo_cache) >= 3:
                # the pool recycles the oldest buffers; drop its cache entry
                wo_cache.pop(next(iter(wo_cache)))
            wos = []
            for k in range(HPC):
                wt = wo_pool.tile([P, 512], F16, tag="wo", name=f"wo{oc}_{k}")
                nc.sync.dma_start(wt[:], wo_d[k, :, oc * 512 : (oc + 1) * 512])
                wos.append(wt)
            wo_cache[oc] = wos
            return wos

        def oproj_chunk(b, oc, prefetch=None, pools=None):
            # out[t, oc] partial: contraction over this core's 5*128 dims
            attnt_sb = state[b, "attnt"]
            wos = load_wo(oc)
            pools = pools or [(mm_pool, "mm")]
            for t in range(QT):
                if t == 1 and prefetch is not None:
                    load_wo(prefetch)
                pool, ptag = pools[t % len(pools)]
                po = pool.tile([P, 512], F32, tag=ptag)
                for k in range(HPC):
                    nc.tensor.matmul(
                        po[:],
                        attnt_sb[:, k, t * P : (t + 1) * P],
                        wos[k][:],
                        start=(k == 0),
                        stop=(k == HPC - 1),
                    )
                # stage to fp16 on the Scalar engine (Copy shares the act
                # table with Exp, so no table reloads) to keep the DVE free
                ot = out_pool.tile([P, 512], F16, tag="ot")
                nc.scalar.activation(
                    ot[:], po[:], mybir.ActivationFunctionType.Copy
                )
                nc.sync.dma_start(
                    out_d[b * QT + t, :, oc * 512 : (oc + 1) * 512], ot[:]
                )
                yield

        def drain(gens):
            for g in gens:
                for _ in g:
                    pass

        def interleave(a_gens, b_gens, ratio):
            """Step generator stream a, inserting `ratio` steps of stream b
            after each a-step. Instruction-level pipelining: b's big dense
            matmuls fill a's dependency stalls so the PE never idles long
            enough for HAM to re-throttle."""
            bi = 0
            for g in a_gens:
                for _ in g:
                    n = 0
                    while n < ratio and bi < len(b_gens):
                        try:
                            next(b_gens[bi])
                            n += 1
                        except StopIteration:
                            bi += 1
            drain(b_gens[bi:])

        # ---- software pipeline: keep the PE stream dense so HAM stays warm
        prefetch_ws(0, 0)
        load_xt(0)
        drain([qkv_mtile(0, m) for m in range(MQKV)])
        load_xt(1)
        interleave(
            [attn_stream(0, nc.vector)],
            [qkv_mtile(1, m) for m in range(MQKV)],
            ratio=3,
        )
        interleave(
            [attn_stream(1, nc.vector)],
            [
                oproj_chunk(0, oc, prefetch=(oc + 1) if oc + 1 < cfg.OC else 0)
                for oc in range(cfg.OC)
            ],
            ratio=1,
        )
        drain(
            [
                oproj_chunk(
                    1,
                    oc,
                    prefetch=(oc + 1) if oc + 1 < cfg.OC else None,
                    pools=[(mm_pool, "mm"), (sc_pool, "sc")],
                )
                for oc in range(cfg.OC)
            ]
        )

    nc.compile()
    return nc


def prep_inputs(hidden_states, W_pack, W_o, attention_mask, cfg: Cfg = FULL):
    """Shard + lay out the full inputs for the 8 cores. Returns in_maps."""
    B, S, KT, HPC = cfg.B, cfg.S, cfg.KT, cfg.HPC
    H = cfg.H
    hs = np.asarray(hidden_states)
    wp = np.asarray(W_pack)
    wo = np.asarray(W_o)
    am = np.asarray(attention_mask)

    # x^T layout [B, 128, KT, S]: xt[b, p, k, t] = hs[b, t, k*128 + p]
    xt = np.ascontiguousarray(
        hs.reshape(B, S, KT, P).transpose(0, 3, 2, 1).astype(np.float16)
    )

    # alibi slopes from the mask: mask[h, q, k] = causal + slope_h * k
    slopes = am[:, -1, 1].astype(np.float64)  # mask[h, S-1, 1] = slope_h

    # q-position ramp, identical in every partition (exact in fp16)
    qrbc = np.tile(np.arange(S, dtype=np.float16)[None, :], (P, 1))

    kvec = np.arange(P, dtype=np.float64)
    in_maps = []
    for c in range(cfg.n_cores):
        heads = range(c * HPC, (c + 1) * HPC)
        # W_pack^T strips: m-tiles [q0..q4, k0..k4, v0..v4] for this core's heads
        rows = []
        for sec in range(3):  # q, k, v blocks of W_pack
            for h in heads:
                r0 = sec * H + h * P
                rows.append(wp[r0 : r0 + P, :])  # [128, H]
        # strip[m, p, k, j] = W_pack[row_j, k*128 + p]
        ws = np.stack(
            [r.T.reshape(KT, P, P).transpose(1, 0, 2) for r in rows]
        ).astype(np.float16)

        # W_o^T strip: wo_c[k, p, o] = W_o[o, c*HPC*128 + k*128 + p]
        wo_c = np.ascontiguousarray(
            wo[:, c * HPC * P : (c + 1) * HPC * P].T.reshape(HPC, P, H)
        ).astype(np.float16)

        # exp bias table [128, HPC*QT]: col hh*QT + i -> slope*(i*128+k) - lnPS
        bias = np.empty((P, HPC * cfg.QT), dtype=np.float32)
        # stabilizer slope column: -slope_h * sqrt(d), replicated down partitions
        slcp = np.empty((P, HPC), dtype=np.float32)
        for hh, h in enumerate(heads):
            for i in range(cfg.QT):
                bias[:, hh * cfg.QT + i] = (
                    slopes[h] * (i * P + kvec) - LN_PSCALE
                ).astype(np.float32)
            slcp[:, hh] = np.float32(-slopes[h] * math.sqrt(128.0))

        in_maps.append(
            {
                "xt": xt,
                "wqkv": np.ascontiguousarray(ws),
                "wo": wo_c,
                "bias": bias,
                "qrbc": qrbc,
                "slcp": slcp,
            }
        )
    return in_maps


_CACHE = {}


def _get_nc(cfg: Cfg = FULL) -> bass.Bass:
    if cfg not in _CACHE:
        _CACHE[cfg] = build_nc(cfg)
    return _CACHE[cfg]


def run(hidden_states, W_pack, W_o, attention_mask, cfg: Cfg = FULL, **kw):
    nc = _get_nc(cfg)
    in_maps = prep_inputs(hidden_states, W_pack, W_o, attention_mask, cfg)
    res = run_bass_kernel_spmd(nc, in_maps, core_ids=list(range(cfg.n_cores)), **kw)
    # sum the per-core partials (fp16 -> fp32), unshard to [B, S, H]
    acc = np.zeros((cfg.B * cfg.QT, P, cfg.H), dtype=np.float32)
    for r in res.results:
        acc += r["out"].astype(np.float32)
    out = acc.reshape(cfg.B, cfg.S, cfg.H)
    return out, res


def kernel(hidden_states, W_pack, W_o, attention_mask):
    out, _ = run(hidden_states, W_pack, W_o, attention_mask)
    return out.astype(np.float32)

